# revision 7
# baseline (speedup 1.0000x reference)
"""Trainium2 Bass kernel for CustomMHA (bsz=4, seq=2048, embd=1024, nhead=16).

The reference's "buggy" head split
    q = query.reshape(T, B*H, hd).swapaxes(0, 1)
is equivalent to de-interleaving rows of the (B*T, E) 2-D view mod 4 into 4
row-groups rb, then a standard 16-head split of the 1024 columns within each
group.  The 64 (rb, cb) heads are sharded 8 per core: core c = rb*2 + half
owns row-group rb and columns cols = [half*512, half*512+512).

Per core (matmuls in float32r = fp32 RNE-rounded to 11 mantissa bits, 1
cycle/row on the PE; inputs pre-rounded on the host and DMAed straight into
f32r tiles):
  K_T[c,s]  = sum_e kwT[e,c] * keyT[e,s] + kb[c]     (c on partitions)
  V[s,c]    = sum_e valueT[e,s] * vw[c,e] + vb[c]    (s on partitions,
              65-wide per-head blocks whose last column is 1.0)
  per head pair (A=2ct, B=2ct+1), streaming over s-tiles:
    S_T[s,t] = sum_d K_T[hd,s] * qT[hd,t]   (row-packed pair, no hd scaling)
    P~ = exp(S_T)  -> DRAM unnormalized     (s,t layout; host transposes)
    pso[0:64]  = sum_s V[s,d] * P~[s,t]     (unnormalized O~)
    pso[64]    = sum_s P~[s,t]              (denominator, via the ones col)
    r = 1/pso[64] -> DRAM (host multiplies attn); O = O~ * r on device
  outp[t,j] = sum_c O[c,t] * w2T[c,j]       (partial; host sums core pairs)
"""
import sys

sys.path.insert(0, "/opt/trn_rl_repo")

import numpy as np

BSZ, SEQ, EMBD, NHEAD = 4, 2048, 1024, 16
HD = EMBD // NHEAD          # 64
CLOC = 512                  # local columns per core (8 heads)
NCT = CLOC // 128           # 4
NET = EMBD // 128           # 8
NST = SEQ // 128            # 16
TBLK = 512
NTB = SEQ // TBLK           # 4
NHL = 8                     # local heads per core
VW = HD + 1                 # 65: per-head V block width (ones column last)

_BASS = None


def _round_f32r(x):
    """RNE to 11 mantissa bits == hardware f32r rounding (verified on HW)."""
    xi = np.ascontiguousarray(x, dtype=np.float32).view(np.uint32).astype(np.uint64)
    shift = 12
    bias = ((xi >> shift) & 1) + (1 << (shift - 1)) - 1
    xi = (xi + bias) >> shift << shift
    return xi.astype(np.uint32).view(np.float32)


def _build_bass():
    import concourse.mybir as mybir
    import concourse.tile as tile
    import concourse.bacc as bacc

    f32 = mybir.dt.float32
    f32r = mybir.dt.float32r
    AF = mybir.ActivationFunctionType
    Alu = mybir.AluOpType

    nc = bacc.Bacc("TRN2", target_bir_lowering=False, debug=False)

    qT_d = nc.declare_dram_parameter("qT", [CLOC, SEQ], f32r, isOutput=False)
    keyT_d = nc.declare_dram_parameter("keyT", [EMBD, SEQ], f32r, isOutput=False)
    valueT_d = nc.declare_dram_parameter("valueT", [EMBD, SEQ], f32r, isOutput=False)
    kwT_d = nc.declare_dram_parameter("kwT", [EMBD, CLOC], f32r, isOutput=False)
    vwT_d = nc.declare_dram_parameter("vwT", [EMBD, CLOC], f32r, isOutput=False)
    w2T_d = nc.declare_dram_parameter("w2T", [CLOC, EMBD], f32r, isOutput=False)
    kb_d = nc.declare_dram_parameter("kb", [128, NCT], f32, isOutput=False)
    vb_d = nc.declare_dram_parameter("vb", [1, CLOC], f32, isOutput=False)
    attn_d = nc.declare_dram_parameter("attn_t", [NHL, SEQ, SEQ], f32, isOutput=True)
    r_d = nc.declare_dram_parameter("rrow", [NHL, SEQ], f32, isOutput=True)
    outp_d = nc.declare_dram_parameter("outp", [SEQ, EMBD], f32, isOutput=True)

    with tile.TileContext(nc) as tc:
        with (
            tc.tile_pool(name="persist", bufs=1) as persist,
            tc.tile_pool(name="wpool", bufs=1) as wpool,
            tc.tile_pool(name="stream", bufs=9) as stream,
            tc.tile_pool(name="qpool", bufs=2) as qpool,
            tc.tile_pool(name="work", bufs=2) as work,
            tc.tile_pool(name="pexpool", bufs=10) as pexpool,
            tc.tile_pool(name="psum_a", bufs=5, space="PSUM") as psum_a,
            tc.tile_pool(name="psum_o", bufs=3, space="PSUM") as psum_o,
        ):
            # ---------------- persistent tiles ----------------
            KT = persist.tile([128, NCT, SEQ], f32r, tag="KT")           # 32 KB/p
            V = persist.tile([128, NST, NHL * VW], f32r, tag="V")        # 32.5 KB/p
            O_sb = persist.tile([128, NCT, SEQ], f32r, tag="O_sb")       # 32 KB/p
            kb_sb = persist.tile([128, NCT], f32, tag="kb")
            vb_sb = persist.tile([128, CLOC], f32, tag="vb")

            nc.sync.dma_start(kb_sb[:], kb_d[:])
            nc.sync.dma_start(vb_sb[:], vb_d[:].to_broadcast((128, CLOC)))

            # ones column of every per-head V block (ACT writes 1.0, f32r)
            ones_view = V[:].rearrange("p s (h x) -> p s h x", x=VW)[:, :, :, HD:VW]
            nc.scalar.activation(
                ones_view,
                vb_sb[:, 0:1, None].to_broadcast(ones_view.shape),
                AF.Copy, bias=1.0, scale=0.0,
            )

            # ---------------- HAM warm-up ----------------
            # fp32r matmuls never register as PE activity for the HAM clock
            # gate (measured: an all-f32r kernel runs at 1.2 GHz forever), but
            # they do keep it warm once a bf16 burst has raised K to 8/8.
            bf16 = mybir.dt.bfloat16
            wu_a = persist.tile([128, 128], bf16, tag="wu_a")
            wu_b = persist.tile([128, 512], bf16, tag="wu_b")
            nc.vector.memset(wu_a[:], 0.25)
            nc.vector.memset(wu_b[:], 0.25)

            def warm_burst(n):
                for _ in range(n):
                    wps = psum_a.tile([128, 512], f32, tag="ps", name="wps")
                    nc.tensor.matmul(wps[:], wu_a[:], wu_b[:],
                                     start=True, stop=True)

            warm_burst(22)

            # ---------------- K projection ----------------
            kwT_r = wpool.tile([128, NET, CLOC], f32r, tag="wr")
            nc.sync.dma_start(kwT_r[:], kwT_d[:].rearrange("(et p) c -> p et c", p=128))

            for sblk in range(4):
                ksl = []
                for et in range(NET):
                    kr = stream.tile([128, 512], f32r, tag="kvslice")
                    nc.sync.dma_start(
                        kr[:], keyT_d[et * 128:(et + 1) * 128, sblk * 512:(sblk + 1) * 512]
                    )
                    ksl.append(kr)
                for ct in range(NCT):
                    ps = psum_a.tile([128, 512], f32, tag="ps")
                    for et in range(NET):
                        nc.tensor.matmul(
                            ps[:],
                            kwT_r[:, et, ct * 128:(ct + 1) * 128],
                            ksl[et][:],
                            start=(et == 0), stop=(et == NET - 1),
                        )
                    # add key bias (per-partition scalar) and round to f32r
                    nc.vector.tensor_scalar(
                        KT[:, ct, sblk * 512:(sblk + 1) * 512], ps[:],
                        kb_sb[:, ct:ct + 1], None, Alu.add,
                    )

            warm_burst(4)

            # ---------------- V projection ----------------
            vwT_r = wpool.tile([128, NET, CLOC], f32r, tag="wr")
            nc.sync.dma_start(vwT_r[:], vwT_d[:].rearrange("(et p) c -> p et c", p=128))

            for stg in range(4):               # groups of 4 s-tiles
                vsl = []
                for et in range(NET):
                    vr = stream.tile([128, 512], f32r, tag="kvslice")
                    nc.sync.dma_start(
                        vr[:], valueT_d[et * 128:(et + 1) * 128, stg * 512:(stg + 1) * 512]
                    )
                    vsl.append(vr)
                for sj in range(4):
                    st = stg * 4 + sj
                    ps = psum_a.tile([128, CLOC], f32, tag="ps")
                    for et in range(NET):
                        nc.tensor.matmul(
                            ps[:],
                            vsl[et][:, sj * 128:(sj + 1) * 128],
                            vwT_r[:, et, :],
                            start=(et == 0), stop=(et == NET - 1),
                        )
                    # bias add + scatter into 65-wide per-head blocks (f32r)
                    vdst = V[:, st, :].rearrange("p (h x) -> p h x", x=VW)[:, :, 0:HD]
                    nc.vector.tensor_tensor(
                        vdst,
                        ps[:].rearrange("p (h x) -> p h x", x=HD),
                        vb_sb[:].rearrange("p (h x) -> p h x", x=HD),
                        Alu.add,
                    )

            # ---------------- attention (head pairs, streaming) ----------------
            for ct in range(NCT):
                warm_burst(3)
                qct = qpool.tile([128, SEQ], f32r, tag="qct")
                nc.sync.dma_start(qct[:], qT_d[ct * 128:(ct + 1) * 128, :])
                hA, hB = 2 * ct, 2 * ct + 1
                for tb in range(NTB):
                    tsl = slice(tb * TBLK, (tb + 1) * TBLK)
                    psoA = psum_o.tile([VW, TBLK], f32, tag="pso")
                    psoB = psum_o.tile([VW, TBLK], f32, tag="pso")
                    for st in range(NST):
                        ssl = slice(st * 128, (st + 1) * 128)
                        psA = psum_a.tile([128, TBLK], f32, tag="ps")
                        nc.tensor.matmul(
                            psA[:], KT[0:64, ct, ssl], qct[0:64, tsl],
                            start=True, stop=True, tile_position=(0, 0),
                        )
                        psB = psum_a.tile([128, TBLK], f32, tag="ps")
                        nc.tensor.matmul(
                            psB[:], KT[64:128, ct, ssl], qct[64:128, tsl],
                            start=True, stop=True, tile_position=(64, 0),
                        )
                        pexA = pexpool.tile([128, TBLK], f32r, tag="pex")
                        nc.scalar.activation(pexA[:], psA[:], AF.Exp)
                        nc.sync.dma_start(attn_d[hA, ssl, tsl], pexA[:].bitcast(f32))
                        nc.tensor.matmul(
                            psoA[:], V[:, st, hA * VW:(hA + 1) * VW], pexA[:],
                            start=(st == 0), stop=(st == NST - 1),
                        )
                        pexB = pexpool.tile([128, TBLK], f32r, tag="pex")
                        nc.scalar.activation(pexB[:], psB[:], AF.Exp)
                        nc.gpsimd.dma_start(attn_d[hB, ssl, tsl], pexB[:].bitcast(f32))
                        nc.tensor.matmul(
                            psoB[:], V[:, st, hB * VW:(hB + 1) * VW], pexB[:],
                            start=(st == 0), stop=(st == NST - 1),
                        )
                    for hp, pso in ((0, psoA), (1, psoB)):
                        hl = 2 * ct + hp
                        prange = slice(hp * 64, hp * 64 + 64)
                        r_row = work.tile([1, TBLK], f32, tag="rrow")
                        nc.vector.reciprocal(r_row[:], pso[HD:VW, :])
                        nc.scalar.dma_start(r_d[hl, tsl], r_row[:])
                        R = work.tile([64, TBLK], f32, tag="R")
                        nc.gpsimd.partition_broadcast(R[:], r_row[:])
                        nc.vector.tensor_tensor(
                            O_sb[prange, ct, tsl], pso[0:HD, :], R[:], Alu.mult
                        )

            warm_burst(4)

            # ---------------- output projection ----------------
            w2T_r = wpool.tile([128, NCT, EMBD], f32r, tag="wr")
            nc.sync.dma_start(w2T_r[:], w2T_d[:].rearrange("(ct p) j -> p ct j", p=128))

            for tt in range(NST):
                osb = work.tile([128, EMBD], f32, tag="osb")
                for jb in range(2):
                    ps = psum_a.tile([128, 512], f32, tag="ps")
                    for ct in range(NCT):
                        nc.tensor.matmul(
                            ps[:],
                            O_sb[:, ct, tt * 128:(tt + 1) * 128],
                            w2T_r[:, ct, jb * 512:(jb + 1) * 512],
                            start=(ct == 0), stop=(ct == NCT - 1),
                        )
                    nc.vector.tensor_copy(osb[:, jb * 512:(jb + 1) * 512], ps[:])
                nc.sync.dma_start(outp_d[tt * 128:(tt + 1) * 128, :], osb[:])

    nc.compile()
    return nc


def _get_bass():
    global _BASS
    if _BASS is None:
        _BASS = _build_bass()
    return _BASS


_last_in_maps = None


def kernel(query, key, value, key_w, key_b, value_w, value_b, out_w, out_b,
           nhead=16, **_unused):
    from concourse.bass_utils import run_bass_kernel_spmd

    q = np.ascontiguousarray(np.asarray(query, dtype=np.float32))
    k = np.ascontiguousarray(np.asarray(key, dtype=np.float32))
    v = np.ascontiguousarray(np.asarray(value, dtype=np.float32))
    kw = np.asarray(key_w, dtype=np.float32)
    kb = np.asarray(key_b, dtype=np.float32)
    vw = np.asarray(value_w, dtype=np.float32)
    vb = np.asarray(value_b, dtype=np.float32)
    ow = np.asarray(out_w, dtype=np.float32)
    ob = np.asarray(out_b, dtype=np.float32)

    Q3 = q.reshape(SEQ, 4, EMBD)
    K3 = k.reshape(SEQ, 4, EMBD)
    V3 = v.reshape(SEQ, 4, EMBD)

    in_maps = []
    for core in range(8):
        rb, half = core // 2, core % 2
        cols = slice(half * CLOC, half * CLOC + CLOC)
        in_maps.append({
            "qT": _round_f32r(Q3[:, rb, cols].T),
            "keyT": _round_f32r(K3[:, rb, :].T),
            "valueT": _round_f32r(V3[:, rb, :].T),
            "kwT": _round_f32r(kw[cols, :].T),
            "vwT": _round_f32r(vw[cols, :].T),
            "w2T": _round_f32r(ow[:, cols].T),
            "kb": np.ascontiguousarray(kb[cols].reshape(NCT, 128).T),
            "vb": vb[cols].reshape(1, CLOC).copy(),
        })

    global _last_in_maps
    _last_in_maps = in_maps
    nc = _get_bass()
    res = run_bass_kernel_spmd(nc, in_maps, list(range(8)))

    out2d = np.empty((BSZ * SEQ, EMBD), np.float32)
    attn = np.empty((BSZ * NHEAD, SEQ, SEQ), np.float32)
    for rb in range(4):
        p0 = res.results[2 * rb]["outp"]
        p1 = res.results[2 * rb + 1]["outp"]
        out2d[rb::4] = p0 + p1 + ob[None, :]
        for half in range(2):
            at = res.results[2 * rb + half]["attn_t"]
            rr = res.results[2 * rb + half]["rrow"]
            for hl in range(NHL):
                bh = rb * 16 + half * 8 + hl
                # transpose (s,t)->(t,s) and normalize in one pass
                np.multiply(at[hl].T, rr[hl][:, None], out=attn[bh])
    out = out2d.reshape(BSZ, SEQ, EMBD)
    return out, attn


# revision 8
# speedup vs baseline: 1.4602x; 1.4602x over previous
"""Trainium2 Bass kernel for CustomMHA (bsz=4, seq=2048, embd=1024, nhead=16).

The reference's "buggy" head split
    q = query.reshape(T, B*H, hd).swapaxes(0, 1)
is equivalent to de-interleaving rows of the (B*T, E) 2-D view mod 4 into 4
row-groups rb, then a standard 16-head split of the 1024 columns within each
group.  The 64 (rb, cb) heads are sharded 8 per core: core c = rb*2 + half
owns row-group rb and columns cols = [half*512, half*512+512).

Per core:
  K_T[c,s]  = sum_e kwT[e,c] * keyT[e,s] + kb[c]    (float32r matmuls; then
              split hi/lo into a [kh;kl] bf16 stack per head)
  V[s,c]    = sum_e valueT[e,s] * vw[c,e] + vb[c]   (bf16, 65-wide per-head
              blocks whose last column is 1.0)
  per head, streaming over s-tiles (all bf16, which keeps the PE HAM-warm —
  fp32r matmuls never register as PE activity for the clock gate):
    S_T[s,t] = [kh;kl].T @ [qh;ql] + [kh;kl].T @ [ql;qh]   (2 matmuls ==
               exact (kh+kl)@(qh+ql), so logits are fp32-quality)
    P~ = exp(S_T) in bf16  -> DRAM unnormalized  (s,t; host transposes)
    pso[0:64] = sum_s V[s,d] * P~[s,t]  (unnormalized O~)
    pso[64]   = sum_s P~[s,t]           (denominator via the ones column)
    r = 1/pso[64] -> DRAM (host multiplies attn); O = O~ * r on device
  outp[t,j] = sum_c O[c,t] * w2T[c,j]   (f32r; partial, host sums core pairs)
"""
import sys

sys.path.insert(0, "/opt/trn_rl_repo")

import numpy as np
import ml_dtypes

BSZ, SEQ, EMBD, NHEAD = 4, 2048, 1024, 16
HD = EMBD // NHEAD          # 64
CLOC = 512                  # local columns per core (8 heads)
NCT = CLOC // 128           # 4
NET = EMBD // 128           # 8
NST = SEQ // 128            # 16
TBLK = 512
NTB = SEQ // TBLK           # 4
NHL = 8                     # local heads per core
VW = HD + 1                 # 65: per-head V block width (ones column last)

_BASS = None


def _round_f32r(x):
    """RNE to 11 mantissa bits == hardware f32r rounding (verified on HW)."""
    xi = np.ascontiguousarray(x, dtype=np.float32).view(np.uint32).astype(np.uint64)
    shift = 12
    bias = ((xi >> shift) & 1) + (1 << (shift - 1)) - 1
    xi = (xi + bias) >> shift << shift
    return xi.astype(np.uint32).view(np.float32)


def _build_bass():
    import concourse.mybir as mybir
    import concourse.tile as tile
    import concourse.bacc as bacc

    f32 = mybir.dt.float32
    f32r = mybir.dt.float32r
    bf16 = mybir.dt.bfloat16
    AF = mybir.ActivationFunctionType
    Alu = mybir.AluOpType

    nc = bacc.Bacc("TRN2", target_bir_lowering=False, debug=False)

    q1_d = nc.declare_dram_parameter("q1", [NHL, 128, SEQ], bf16, isOutput=False)
    q2_d = nc.declare_dram_parameter("q2", [NHL, 128, SEQ], bf16, isOutput=False)
    keyT_d = nc.declare_dram_parameter("keyT", [EMBD, SEQ], f32r, isOutput=False)
    valueT_d = nc.declare_dram_parameter("valueT", [EMBD, SEQ], f32r, isOutput=False)
    kwT_d = nc.declare_dram_parameter("kwT", [EMBD, CLOC], f32r, isOutput=False)
    vwT_d = nc.declare_dram_parameter("vwT", [EMBD, CLOC], f32r, isOutput=False)
    w2T_d = nc.declare_dram_parameter("w2T", [CLOC, EMBD], f32r, isOutput=False)
    kb_d = nc.declare_dram_parameter("kb", [128, NCT], f32, isOutput=False)
    vb_d = nc.declare_dram_parameter("vb", [1, CLOC], f32, isOutput=False)
    attn_d = nc.declare_dram_parameter("attn_t", [NHL, SEQ, SEQ], bf16, isOutput=True)
    r_d = nc.declare_dram_parameter("rrow", [NHL, SEQ], f32, isOutput=True)
    outp_d = nc.declare_dram_parameter("outp", [SEQ, EMBD], f32, isOutput=True)

    with tile.TileContext(nc) as tc:
        with (
            tc.tile_pool(name="persist", bufs=1) as persist,
            tc.tile_pool(name="wpool", bufs=1) as wpool,
            tc.tile_pool(name="stream", bufs=9) as stream,
            tc.tile_pool(name="qpool", bufs=2) as qpool,
            tc.tile_pool(name="work", bufs=2) as work,
            tc.tile_pool(name="pexpool", bufs=10) as pexpool,
            tc.tile_pool(name="psum_a", bufs=5, space="PSUM") as psum_a,
            tc.tile_pool(name="psum_o", bufs=3, space="PSUM") as psum_o,
        ):
            # ---------------- persistent tiles ----------------
            # KT rows 0:64 = bf16 hi of K_T per head, rows 64:128 = bf16 lo
            KT = persist.tile([128, NHL, SEQ], bf16, tag="KT")           # 32 KB/p
            V = persist.tile([128, NST, NHL * VW], bf16, tag="V")        # 16.3 KB/p
            O_sb = persist.tile([128, NCT, SEQ], f32r, tag="O_sb")       # 32 KB/p
            kb_sb = persist.tile([128, NCT], f32, tag="kb")
            vb_sb = persist.tile([128, CLOC], f32, tag="vb")

            nc.sync.dma_start(kb_sb[:], kb_d[:])
            nc.sync.dma_start(vb_sb[:], vb_d[:].to_broadcast((128, CLOC)))

            # ones column of every per-head V block (ACT writes 1.0, bf16)
            ones_view = V[:].rearrange("p s (h x) -> p s h x", x=VW)[:, :, :, HD:VW]
            nc.scalar.activation(
                ones_view,
                vb_sb[:, 0:1, None].to_broadcast(ones_view.shape),
                AF.Copy, bias=1.0, scale=0.0,
            )

            # ---------------- HAM warm-up ----------------
            # fp32r matmuls never raise the HAM clock gate to K=8/8 (an
            # all-f32r kernel runs at 1.2 GHz forever); bf16 bursts do.
            wu_a = persist.tile([128, 128], bf16, tag="wu_a")
            wu_b = persist.tile([128, 512], bf16, tag="wu_b")
            nc.vector.memset(wu_a[:], 0.25)
            nc.vector.memset(wu_b[:], 0.25)

            def warm_burst(n):
                for _ in range(n):
                    wps = psum_a.tile([128, 512], f32, tag="ps", name="wps")
                    nc.tensor.matmul(wps[:], wu_a[:], wu_b[:],
                                     start=True, stop=True)

            warm_burst(22)

            # ---------------- K projection (f32r) + hi/lo bf16 split -------
            kwT_r = wpool.tile([128, NET, CLOC], f32r, tag="wr")
            nc.sync.dma_start(kwT_r[:], kwT_d[:].rearrange("(et p) c -> p et c", p=128))

            for sblk in range(4):
                ssl = slice(sblk * 512, (sblk + 1) * 512)
                ksl = []
                for et in range(NET):
                    kr = stream.tile([128, 512], f32r, tag="kvslice")
                    nc.sync.dma_start(kr[:], keyT_d[et * 128:(et + 1) * 128, ssl])
                    ksl.append(kr)
                for ct in range(NCT):
                    ps = psum_a.tile([128, 512], f32, tag="ps")
                    for et in range(NET):
                        nc.tensor.matmul(
                            ps[:],
                            kwT_r[:, et, ct * 128:(ct + 1) * 128],
                            ksl[et][:],
                            start=(et == 0), stop=(et == NET - 1),
                        )
                    for hp in range(2):
                        h = 2 * ct + hp
                        pr = slice(hp * 64, hp * 64 + 64)
                        kt0 = work.tile([64, 512], f32, tag="kt0")
                        nc.vector.tensor_scalar(
                            kt0[:], ps[pr, :],
                            kb_sb[pr, ct:ct + 1], None, Alu.add,
                        )
                        nc.vector.tensor_copy(KT[0:64, h, ssl], kt0[:])
                        nc.vector.tensor_tensor(
                            KT[64:128, h, ssl], kt0[:], KT[0:64, h, ssl],
                            Alu.subtract,
                        )

            # ---------------- V projection (f32r -> bf16) ----------------
            vwT_r = wpool.tile([128, NET, CLOC], f32r, tag="wr")
            nc.sync.dma_start(vwT_r[:], vwT_d[:].rearrange("(et p) c -> p et c", p=128))

            for stg in range(4):               # groups of 4 s-tiles
                vsl = []
                for et in range(NET):
                    vr = stream.tile([128, 512], f32r, tag="kvslice")
                    nc.sync.dma_start(
                        vr[:], valueT_d[et * 128:(et + 1) * 128,
                                        stg * 512:(stg + 1) * 512]
                    )
                    vsl.append(vr)
                for sj in range(4):
                    st = stg * 4 + sj
                    ps = psum_a.tile([128, CLOC], f32, tag="ps")
                    for et in range(NET):
                        nc.tensor.matmul(
                            ps[:],
                            vsl[et][:, sj * 128:(sj + 1) * 128],
                            vwT_r[:, et, :],
                            start=(et == 0), stop=(et == NET - 1),
                        )
                    # bias add + scatter into 65-wide per-head blocks (bf16)
                    vdst = V[:, st, :].rearrange("p (h x) -> p h x", x=VW)[:, :, 0:HD]
                    nc.vector.tensor_tensor(
                        vdst,
                        ps[:].rearrange("p (h x) -> p h x", x=HD),
                        vb_sb[:].rearrange("p (h x) -> p h x", x=HD),
                        Alu.add,
                    )

            warm_burst(16)

            # ---------------- attention (streaming, bf16) ----------------
            for ct in range(NCT):
                q1c = qpool.tile([128, 2, SEQ], bf16, tag="q1c")
                nc.sync.dma_start(
                    q1c[:], q1_d[2 * ct:2 * ct + 2].rearrange("h p t -> p h t"))
                q2c = qpool.tile([128, 2, SEQ], bf16, tag="q2c")
                nc.sync.dma_start(
                    q2c[:], q2_d[2 * ct:2 * ct + 2].rearrange("h p t -> p h t"))
                for tb in range(NTB):
                    tsl = slice(tb * TBLK, (tb + 1) * TBLK)
                    psoA = psum_o.tile([VW, TBLK], f32, tag="pso")
                    psoB = psum_o.tile([VW, TBLK], f32, tag="pso")
                    for st in range(NST):
                        ssl = slice(st * 128, (st + 1) * 128)
                        for hp, pso in ((0, psoA), (1, psoB)):
                            hl = 2 * ct + hp
                            ps = psum_a.tile([128, TBLK], f32, tag="ps", name="s_ps")
                            nc.tensor.matmul(
                                ps[:], KT[:, hl, ssl], q1c[:, hp, tsl],
                                start=True, stop=False,
                            )
                            nc.tensor.matmul(
                                ps[:], KT[:, hl, ssl], q2c[:, hp, tsl],
                                start=False, stop=True,
                            )
                            pex = pexpool.tile([128, TBLK], bf16, tag="pex")
                            nc.scalar.activation(pex[:], ps[:], AF.Exp)
                            if hp == 0:
                                nc.sync.dma_start(attn_d[hl, ssl, tsl], pex[:])
                            else:
                                nc.gpsimd.dma_start(attn_d[hl, ssl, tsl], pex[:])
                            nc.tensor.matmul(
                                pso[:], V[:, st, hl * VW:(hl + 1) * VW], pex[:],
                                start=(st == 0), stop=(st == NST - 1),
                            )
                    for hp, pso in ((0, psoA), (1, psoB)):
                        hl = 2 * ct + hp
                        prange = slice(hp * 64, hp * 64 + 64)
                        r_row = work.tile([1, TBLK], f32, tag="rrow")
                        nc.vector.reciprocal(r_row[:], pso[HD:VW, :])
                        nc.scalar.dma_start(r_d[hl, tsl], r_row[:])
                        R = work.tile([64, TBLK], f32, tag="R")
                        nc.gpsimd.partition_broadcast(R[:], r_row[:])
                        nc.vector.tensor_tensor(
                            O_sb[prange, ct, tsl], pso[0:HD, :], R[:], Alu.mult
                        )

            warm_burst(16)

            # ---------------- output projection (f32r) ----------------
            w2T_r = wpool.tile([128, NCT, EMBD], f32r, tag="wr")
            nc.sync.dma_start(w2T_r[:], w2T_d[:].rearrange("(ct p) j -> p ct j", p=128))

            for tt in range(NST):
                osb = work.tile([128, EMBD], f32, tag="osb")
                for jb in range(2):
                    ps = psum_a.tile([128, 512], f32, tag="ps")
                    for ct in range(NCT):
                        nc.tensor.matmul(
                            ps[:],
                            O_sb[:, ct, tt * 128:(tt + 1) * 128],
                            w2T_r[:, ct, jb * 512:(jb + 1) * 512],
                            start=(ct == 0), stop=(ct == NCT - 1),
                        )
                    nc.vector.tensor_copy(osb[:, jb * 512:(jb + 1) * 512], ps[:])
                nc.sync.dma_start(outp_d[tt * 128:(tt + 1) * 128, :], osb[:])

    nc.compile()
    return nc


def _get_bass():
    global _BASS
    if _BASS is None:
        _BASS = _build_bass()
    return _BASS


_last_in_maps = None


def kernel(query, key, value, key_w, key_b, value_w, value_b, out_w, out_b,
           nhead=16, **_unused):
    from concourse.bass_utils import run_bass_kernel_spmd

    q = np.ascontiguousarray(np.asarray(query, dtype=np.float32))
    k = np.ascontiguousarray(np.asarray(key, dtype=np.float32))
    v = np.ascontiguousarray(np.asarray(value, dtype=np.float32))
    kw = np.asarray(key_w, dtype=np.float32)
    kb = np.asarray(key_b, dtype=np.float32)
    vw = np.asarray(value_w, dtype=np.float32)
    vb = np.asarray(value_b, dtype=np.float32)
    ow = np.asarray(out_w, dtype=np.float32)
    ob = np.asarray(out_b, dtype=np.float32)

    Q3 = q.reshape(SEQ, 4, EMBD)
    K3 = k.reshape(SEQ, 4, EMBD)
    V3 = v.reshape(SEQ, 4, EMBD)

    in_maps = []
    for core in range(8):
        rb, half = core // 2, core % 2
        cols = slice(half * CLOC, half * CLOC + CLOC)
        # per-head hi/lo bf16 split of q, stacked [qh;ql] and [ql;qh]
        qT = np.ascontiguousarray(Q3[:, rb, cols].T)       # (512, 2048)
        qh = qT.astype(ml_dtypes.bfloat16)
        ql = (qT - qh.astype(np.float32)).astype(ml_dtypes.bfloat16)
        qh = qh.reshape(NHL, HD, SEQ)
        ql = ql.reshape(NHL, HD, SEQ)
        q1 = np.concatenate([qh, ql], axis=1)              # (8, 128, 2048)
        q2 = np.concatenate([ql, qh], axis=1)
        in_maps.append({
            "q1": np.ascontiguousarray(q1),
            "q2": np.ascontiguousarray(q2),
            "keyT": _round_f32r(K3[:, rb, :].T),
            "valueT": _round_f32r(V3[:, rb, :].T),
            "kwT": _round_f32r(kw[cols, :].T),
            "vwT": _round_f32r(vw[cols, :].T),
            "w2T": _round_f32r(ow[:, cols].T),
            "kb": np.ascontiguousarray(kb[cols].reshape(NCT, 128).T),
            "vb": vb[cols].reshape(1, CLOC).copy(),
        })

    global _last_in_maps
    _last_in_maps = in_maps
    nc = _get_bass()
    res = run_bass_kernel_spmd(nc, in_maps, list(range(8)))

    out2d = np.empty((BSZ * SEQ, EMBD), np.float32)
    attn = np.empty((BSZ * NHEAD, SEQ, SEQ), np.float32)
    for rb in range(4):
        p0 = res.results[2 * rb]["outp"]
        p1 = res.results[2 * rb + 1]["outp"]
        out2d[rb::4] = p0 + p1 + ob[None, :]
        for half in range(2):
            at = res.results[2 * rb + half]["attn_t"]
            rr = res.results[2 * rb + half]["rrow"]
            for hl in range(NHL):
                bh = rb * 16 + half * 8 + hl
                # upcast bf16, transpose (s,t)->(t,s), normalize in one pass
                np.multiply(at[hl].T.astype(np.float32), rr[hl][:, None],
                            out=attn[bh])
    out = out2d.reshape(BSZ, SEQ, EMBD)
    return out, attn
